# revision 12
# baseline (speedup 1.0000x reference)
"""Trainium2 Bass kernel: CNN encoder + 2-layer LSTM decoder + attention.

Sharding: data-parallel over batch (B=8) across 8 NeuronCores; all weights
replicated. Each core processes one batch element end-to-end.

Key layout ideas (per core, batch=1):
  * Everything lives "transposed": channels/hidden on the 128-partition axis.
  * Convs are 3-tap shifted matmuls accumulating in PSUM.
  * LSTM recurrence: whh stationary ([K=128,M=128] tiles, bf16 -> FWL),
    h as the N=1 moving operand. Gates land as [128, 16] columns in PSUM,
    which IS the transposed layout the next step needs - no transposes.
  * Attention collapses: softmax(dec_p[t]+enc_p[s]+b) over s == softmax(enc_p)
    (softmax is shift-invariant), so attn weights are identical for every t and
    context is one row broadcast over T. Computed exactly that way.
"""

import numpy as np
import ml_dtypes
from contextlib import ExitStack

import concourse.bass as bass
import concourse.bacc as bacc
import concourse.tile as tile
from concourse import mybir

F32 = mybir.dt.float32
BF16 = mybir.dt.bfloat16
AFT = mybir.ActivationFunctionType

EMBED, HID, VOCAB = 256, 512, 10
B, S, T = 8, 1024, 256
SENC = 256
G = 4 * HID          # 2048 gates
NG = G // 128        # 16 gate chunks
KH = HID // 128      # 4 hidden chunks
KE = EMBED // 128    # 2 embed chunks
L1, L2, L3 = 1024, 512, 256   # conv input lengths


def _chunks(n, c=128):
    out = []
    o = 0
    while o < n:
        out.append((o, min(c, n - o)))
        o += c
    return out


def build_bass(nsteps=T):
    """Build the per-core Bass program (SPMD: same program, per-core data)."""
    nc = bacc.Bacc("TRN2")

    def inp(name, shape, dtype=F32):
        return nc.dram_tensor(name, list(shape), dtype, kind="ExternalInput")

    xtgt = inp("xtgt", [128, KE, nsteps])
    wih0 = inp("wih0", [128, KE, G])
    bls0 = inp("bls0", [128, NG])
    whh0 = inp("whh0", [128, KH, G], BF16)
    xsrc = inp("xsrc", [128, KE, S])
    w1 = inp("w1", [128, 3, KE, 256])
    b1 = inp("b1", [128, 2])
    w2 = inp("w2", [128, 3, KE, 512])
    b2 = inp("b2", [128, 4])
    w3 = inp("w3", [128, 3, KH, 512])
    b3 = inp("b3", [128, 4])
    b3f = inp("b3f", [1, 512])
    wih1 = inp("wih1", [128, KH, G], BF16)
    bls1 = inp("bls1", [128, NG])
    whh1 = inp("whh1", [128, KH, G], BF16)
    wenc = inp("wenc", [128, KH])
    outw = inp("outw", [128, KH, VOCAB])
    outb = inp("outb", [1, VOCAB])

    out = nc.dram_tensor("out", [nsteps, VOCAB], F32, kind="ExternalOutput")
    ctxrow = nc.dram_tensor("ctxrow", [1, HID], F32, kind="ExternalOutput")

    io = locals()
    with tile.TileContext(nc) as tc:
        with ExitStack() as ctx:
            _body(ctx, tc, nsteps, io)
    nc.compile()
    return nc


def _body(ctx, tc, nsteps, io):
    nc = tc.nc

    def dma(out, in_, touch=True):
        # The in-place copy "touch" folds the DMA's multi-queue semaphores
        # into one DVE semaphore: LDWEIGHTS can only carry a single wait.
        nc.sync.dma_start(out=out, in_=in_)
        if touch:
            nc.vector.tensor_copy(out, out)

    consts = ctx.enter_context(tc.tile_pool(name="consts", bufs=1))
    data = ctx.enter_context(tc.tile_pool(name="data", bufs=1))
    step = ctx.enter_context(tc.tile_pool(name="step", bufs=3))
    ps_big = ctx.enter_context(tc.tile_pool(name="ps_big", bufs=2, space="PSUM"))
    ps_xg = ctx.enter_context(tc.tile_pool(name="ps_xg", bufs=2, space="PSUM"))
    ps_g = ctx.enter_context(tc.tile_pool(name="ps_g", bufs=4, space="PSUM"))

    # ---- SBUF const tiles + DMAs (priority order: recurrence-critical first)
    xtgt_sb = consts.tile([128, KE, nsteps], F32)
    dma(out=xtgt_sb, in_=io["xtgt"][:])
    wih0_sb = consts.tile([128, KE, G], F32)
    dma(out=wih0_sb, in_=io["wih0"][:])
    bls0_sb = consts.tile([128, NG], F32)
    dma(out=bls0_sb, in_=io["bls0"][:])
    whh0_sb = consts.tile([128, KH, G], BF16)
    dma(out=whh0_sb, in_=io["whh0"][:])

    xsrc_sb = data.tile([128, KE, S + 2], F32)
    nc.vector.memset(xsrc_sb[:, :, 0:1], 0.0)
    nc.vector.memset(xsrc_sb[:, :, S + 1:S + 2], 0.0)
    dma(out=xsrc_sb[:, :, 1:S + 1], in_=io["xsrc"][:])
    w1_sb = consts.tile([128, 3, KE, 256], F32)
    dma(out=w1_sb, in_=io["w1"][:])
    b1_sb = consts.tile([128, 2], F32)
    dma(out=b1_sb, in_=io["b1"][:])
    w2_sb = consts.tile([128, 3, KE, 512], F32)
    dma(out=w2_sb, in_=io["w2"][:])
    b2_sb = consts.tile([128, 4], F32)
    dma(out=b2_sb, in_=io["b2"][:])
    w3_sb = consts.tile([128, 3, KH, 512], F32)
    dma(out=w3_sb, in_=io["w3"][:])
    b3_sb = consts.tile([128, 4], F32)
    dma(out=b3_sb, in_=io["b3"][:])
    b3f_sb = consts.tile([128, 512], F32)
    dma(out=b3f_sb, in_=io["b3f"][:].to_broadcast([128, 512]))

    wih1_sb = consts.tile([128, KH, G], BF16)
    dma(out=wih1_sb, in_=io["wih1"][:])
    bls1_sb = consts.tile([128, NG], F32)
    dma(out=bls1_sb, in_=io["bls1"][:])
    whh1_sb = consts.tile([128, KH, G], BF16)
    dma(out=whh1_sb, in_=io["whh1"][:])
    wenc_sb = consts.tile([128, KH], F32)
    dma(out=wenc_sb, in_=io["wenc"][:])
    outw_sb = consts.tile([128, KH, VOCAB], F32)
    dma(out=outw_sb, in_=io["outw"][:])
    outb_sb = consts.tile([128, VOCAB], F32)
    dma(out=outb_sb, in_=io["outb"][:].to_broadcast([128, VOCAB]))

    # ---- persistent activations
    xg0T = data.tile([128, NG, nsteps], F32)    # layer0 input gates, transposed
    xg1T = data.tile([128, NG, nsteps], F32)
    H0 = data.tile([128, KH, nsteps], BF16)     # layer0 hidden states (h.T)
    H1 = data.tile([128, KH, nsteps], F32)      # layer1 hidden states (h.T)
    y1 = data.tile([128, KE, L2 + 2], F32)      # conv1 pooled out (+pad cols)
    y2 = data.tile([128, KH, L3 + 2], F32)      # conv2 pooled out (+pad cols)
    encT = data.tile([128, KH, SENC], F32)      # enc.T  [hid, s]
    enc_sb = data.tile([128, SENC // 128, HID], F32)  # enc    [s, hid]

    mm = nc.tensor.matmul

    # ---- xg0 = emb[tgt] @ wih0.T + bias  (transposed: [gate, t])
    for m in range(NG):
        ps = ps_xg.tile([128, nsteps], F32, tag="psxg")
        for k in range(KE):
            mm(ps, wih0_sb[:, k, m * 128:(m + 1) * 128], xtgt_sb[:, k, :],
               start=(k == 0), stop=(k == KE - 1))
        nc.vector.tensor_scalar_add(xg0T[:, m, :], ps, bls0_sb[:, m:m + 1])

    # ---- LSTM step emitter. Gate chunk order is [i, f, o, g] (host permutes
    # the weights), so one sigmoid covers i/f/o and one tanh covers g.
    def make_stream(name, xgT, whh_sb, Hout, h_bf16_direct):
        st = {"c": None, "hb": None}

        def emit_step(t):
            if t == 0:
                gs = xgT[:, :, 0]
            else:
                rhs_src = Hout if h_bf16_direct else st["hb"]
                ps = ps_g.tile([128, NG], F32, tag="psg")
                for m in range(NG):
                    for k in range(KH):
                        rhs = (rhs_src[:, k, t - 1:t] if h_bf16_direct
                               else rhs_src[:, k:k + 1])
                        mm(ps[:, m:m + 1],
                           whh_sb[:, k, m * 128:(m + 1) * 128], rhs,
                           start=(k == 0), stop=(k == KH - 1))
                gs_t = step.tile([128, NG], F32, tag="gs" + name)
                nc.vector.tensor_add(gs_t, ps, xgT[:, :, t])
                gs = gs_t
            sifo = step.tile([128, 12], F32, tag="sifo" + name)
            nc.scalar.activation(sifo, gs[:, 0:12], AFT.Sigmoid)
            tg = step.tile([128, 4], F32, tag="tg" + name)
            nc.scalar.activation(tg, gs[:, 12:16], AFT.Tanh)
            ig = step.tile([128, 4], F32, tag="ig" + name)
            nc.vector.tensor_mul(ig, sifo[:, 0:4], tg)
            if t == 0:
                c_new = ig
            else:
                fc = step.tile([128, 4], F32, tag="fc" + name)
                nc.vector.tensor_mul(fc, sifo[:, 4:8], st["c"])
                c_new = step.tile([128, 4], F32, tag="cnew" + name)
                nc.vector.tensor_add(c_new, ig, fc)
            tc_ = step.tile([128, 4], F32, tag="tc" + name)
            nc.scalar.activation(tc_, c_new, AFT.Tanh)
            nc.vector.tensor_mul(Hout[:, :, t], sifo[:, 8:12], tc_)
            if not h_bf16_direct and t < nsteps - 1:
                hb = step.tile([128, KH], BF16, tag="hb" + name)
                nc.vector.tensor_copy(hb, Hout[:, :, t])
                st["hb"] = hb
            st["c"] = c_new

        return emit_step

    def emit_xg1_block(t0, tn):
        for m in range(NG):
            ps = ps_xg.tile([128, tn], F32, tag="psxg")
            for k in range(KH):
                mm(ps, wih1_sb[:, k, m * 128:(m + 1) * 128], H0[:, k, t0:t0 + tn],
                   start=(k == 0), stop=(k == KH - 1))
            nc.vector.tensor_scalar_add(xg1T[:, m, t0:t0 + tn], ps,
                                        bls1_sb[:, m:m + 1])

    # Block-interleave the two layers: while layer0 runs block b, layer1 (one
    # block behind) has independent work ready - its matmuls fill the PE gaps
    # left by layer0's activation chains, and vice versa.
    step0 = make_stream("a", xg0T, whh0_sb, H0, h_bf16_direct=True)
    step1 = make_stream("b", xg1T, whh1_sb, H1, h_bf16_direct=False)
    BL = min(32, nsteps)
    blocks = _chunks(nsteps, BL)
    for bi, (t0, tn) in enumerate(blocks):
        for t in range(t0, t0 + tn):
            step0(t)
        if bi >= 1:
            p0, pn = blocks[bi - 1]
            emit_xg1_block(p0, pn)
            for t in range(p0, p0 + pn):
                step1(t)
    p0, pn = blocks[-1]
    emit_xg1_block(p0, pn)
    for t in range(p0, p0 + pn):
        step1(t)

    # ---- CNN encoder (independent; fills PE gaps during the recurrence)
    # conv1: [256,1024] -> relu/pool -> y1 [256, 512]
    nc.vector.memset(y1[:, :, 0:1], 0.0)
    nc.vector.memset(y1[:, :, L2 + 1:L2 + 2], 0.0)
    for m in range(2):
        for lt in range(2):
            ps = ps_big.tile([128, 512], F32, tag="psbig")
            first = True
            for d in range(3):
                for k in range(KE):
                    mm(ps, w1_sb[:, d, k, m * 128:(m + 1) * 128],
                       xsrc_sb[:, k, d + 512 * lt: d + 512 * lt + 512],
                       start=first, stop=(d == 2 and k == KE - 1))
                    first = False
            pr = step.tile([128, 512], F32, tag="convrelu")
            nc.scalar.activation(pr, ps, AFT.Relu, bias=b1_sb[:, m:m + 1])
            nc.vector.tensor_max(y1[:, m, 1 + 256 * lt: 1 + 256 * lt + 256],
                                 pr[:, 0:512:2], pr[:, 1:512:2])
    # conv2: [256,512] -> relu/pool -> y2 [512, 256]
    nc.vector.memset(y2[:, :, 0:1], 0.0)
    nc.vector.memset(y2[:, :, L3 + 1:L3 + 2], 0.0)
    for m in range(4):
        ps = ps_big.tile([128, 512], F32, tag="psbig")
        first = True
        for d in range(3):
            for k in range(KE):
                mm(ps, w2_sb[:, d, k, m * 128:(m + 1) * 128],
                   y1[:, k, d: d + 512],
                   start=first, stop=(d == 2 and k == KE - 1))
                first = False
        pr = step.tile([128, 512], F32, tag="convrelu")
        nc.scalar.activation(pr, ps, AFT.Relu, bias=b2_sb[:, m:m + 1])
        nc.vector.tensor_max(y2[:, m, 1:1 + 256],
                             pr[:, 0:512:2], pr[:, 1:512:2])
    # conv3 orientation A: encT [hid, s] (stationary = weights)
    for m in range(4):
        ps = ps_xg.tile([128, SENC], F32, tag="psxg")
        first = True
        for d in range(3):
            for k in range(KH):
                mm(ps, w3_sb[:, d, k, m * 128:(m + 1) * 128],
                   y2[:, k, d: d + SENC],
                   start=first, stop=(d == 2 and k == KH - 1))
                first = False
        nc.scalar.activation(encT[:, m, :], ps, AFT.Relu, bias=b3_sb[:, m:m + 1])
    # conv3 orientation B: enc [s, hid] (stationary = activations)
    for sc in range(SENC // 128):
        ps = ps_big.tile([128, 512], F32, tag="psbig")
        first = True
        for d in range(3):
            for k in range(KH):
                mm(ps, y2[:, k, d + 128 * sc: d + 128 * sc + 128],
                   w3_sb[:, d, k, :],
                   start=first, stop=(d == 2 and k == KH - 1))
                first = False
        tmpb = step.tile([128, 512], F32, tag="encbias")
        nc.vector.tensor_add(tmpb, ps, b3f_sb)
        nc.scalar.activation(enc_sb[:, sc, :], tmpb, AFT.Relu)

    # ---- attention (collapsed): attn = softmax(enc @ w_enc), ctx = attn @ enc
    epx = step.tile([128, 2], F32, tag="epx")
    for sc in range(2):
        ps = ps_xg.tile([128, 1], F32, tag="psxg")
        for k in range(KH):
            mm(ps, encT[:, k, sc * 128:(sc + 1) * 128], wenc_sb[:, k:k + 1],
               start=(k == 0), stop=(k == KH - 1))
        nc.scalar.activation(epx[:, sc:sc + 1], ps, AFT.Exp)
    ones = consts.tile([128, 1], F32)
    nc.vector.memset(ones, 1.0)
    ps_sum = ps_xg.tile([1, 1], F32, tag="psxg")
    for sc in range(2):
        mm(ps_sum, epx[:, sc:sc + 1], ones, start=(sc == 0), stop=(sc == 1))
    rsum = step.tile([1, 1], F32, tag="rsum")
    nc.vector.reciprocal(rsum, ps_sum)
    ps_ctx = ps_xg.tile([1, HID], F32, tag="psxg")
    for sc in range(2):
        mm(ps_ctx, epx[:, sc:sc + 1], enc_sb[:, sc, :],
           start=(sc == 0), stop=(sc == 1))
    ctx_sb = step.tile([1, HID], F32, tag="ctxsb")
    nc.vector.tensor_scalar_mul(ctx_sb, ps_ctx, rsum)
    nc.sync.dma_start(out=io["ctxrow"][:], in_=ctx_sb)

    # ---- output head: out = H1.T @ outw.T + outb
    for tc0, tlen in _chunks(nsteps):
        ps = ps_xg.tile([128, VOCAB], F32, tag="psxg")
        for k in range(KH):
            mm(ps[:tlen, :], H1[:, k, tc0:tc0 + tlen], outw_sb[:, k, :],
               start=(k == 0), stop=(k == KH - 1))
        osb = step.tile([128, VOCAB], F32, tag="osb")
        nc.vector.tensor_add(osb[:tlen, :], ps[:tlen, :], outb_sb[:tlen, :])
        nc.sync.dma_start(out=io["out"][tc0:tc0 + tlen, :], in_=osb[:tlen, :])


# ---------------------------------------------------------------------------
# Host-side glue
# ---------------------------------------------------------------------------

def _bf16(x):
    return np.ascontiguousarray(x.astype(ml_dtypes.bfloat16))


def _f32(x):
    return np.ascontiguousarray(x.astype(np.float32))


def _chunkT(v, k):
    """[k*128] -> [128, k]  (column j = chunk j)"""
    return np.ascontiguousarray(v.reshape(k, 128).T)


# gate-row permutation: PyTorch order [i, f, g, o] -> kernel order [i, f, o, g]
_GPERM = np.concatenate([np.arange(0, 1024),
                         np.arange(1536, 2048),
                         np.arange(1024, 1536)])


def prepare_inputs(inputs, nsteps=T):
    """Split/transform full inputs into per-core in_maps (list of 8 dicts)."""
    emb = np.asarray(inputs["embedding"], np.float32)
    src = np.asarray(inputs["src"])
    tgt = np.asarray(inputs["tgt"])

    def conv_w(w, kc):  # (cout, cin, 3) -> [128, 3, kc, cout]
        cout = w.shape[0]
        a = w.transpose(1, 2, 0)                      # (cin, 3, cout)
        a = a.reshape(kc, 128, 3, cout)               # (kc, p, 3, cout)
        return np.ascontiguousarray(a.transpose(1, 2, 0, 3).astype(np.float32))

    def lin_wT(w, kc, dtype=np.float32):  # (gout, din) -> [128, kc, gout]
        a = w.T.reshape(kc, 128, w.shape[0])
        a = a.transpose(1, 0, 2)
        if dtype is np.float32:
            return np.ascontiguousarray(a.astype(np.float32))
        return np.ascontiguousarray(a.astype(ml_dtypes.bfloat16))

    shared = {
        "w1": conv_w(np.asarray(inputs["conv1_w"]), KE),
        "b1": _f32(_chunkT(np.asarray(inputs["conv1_b"]), 2)),
        "w2": conv_w(np.asarray(inputs["conv2_w"]), KE),
        "b2": _f32(_chunkT(np.asarray(inputs["conv2_b"]), 4)),
        "w3": conv_w(np.asarray(inputs["conv3_w"]), KH),
        "b3": _f32(_chunkT(np.asarray(inputs["conv3_b"]), 4)),
        "b3f": _f32(np.asarray(inputs["conv3_b"]).reshape(1, 512)),
        "wih0": lin_wT(np.asarray(inputs["lstm0_wih"])[_GPERM], KE),
        "bls0": _f32(_chunkT((np.asarray(inputs["lstm0_bih"])
                             + np.asarray(inputs["lstm0_bhh"]))[_GPERM], NG)),
        "whh0": lin_wT(np.asarray(inputs["lstm0_whh"])[_GPERM], KH, ml_dtypes.bfloat16),
        "wih1": lin_wT(np.asarray(inputs["lstm1_wih"])[_GPERM], KH, ml_dtypes.bfloat16),
        "bls1": _f32(_chunkT((np.asarray(inputs["lstm1_bih"])
                             + np.asarray(inputs["lstm1_bhh"]))[_GPERM], NG)),
        "whh1": lin_wT(np.asarray(inputs["lstm1_whh"])[_GPERM], KH, ml_dtypes.bfloat16),
        "wenc": _f32(_chunkT(np.asarray(inputs["attn_w"])[0, HID:], KH)),
        "outw": np.ascontiguousarray(
            np.asarray(inputs["out_w"]).T.reshape(KH, 128, VOCAB)
            .transpose(1, 0, 2).astype(np.float32)),
        "outb": _f32(np.asarray(inputs["out_b"]).reshape(1, VOCAB)),
    }

    in_maps = []
    for b in range(B):
        xs = emb[src[b]].T            # (256, S)
        xt = emb[tgt[b, :nsteps]].T   # (256, nsteps)
        m = dict(shared)
        m["xsrc"] = np.ascontiguousarray(
            xs.reshape(KE, 128, S).transpose(1, 0, 2).astype(np.float32))
        m["xtgt"] = np.ascontiguousarray(
            xt.reshape(KE, 128, nsteps).transpose(1, 0, 2).astype(np.float32))
        in_maps.append(m)
    return in_maps


_PROG = {}


def _get_prog(nsteps=T):
    if nsteps not in _PROG:
        _PROG[nsteps] = build_bass(nsteps)
    return _PROG[nsteps]


def kernel(**inputs):
    from concourse.bass_utils import run_bass_kernel_spmd
    nc = _get_prog()
    in_maps = prepare_inputs(inputs)
    res = run_bass_kernel_spmd(nc, in_maps, core_ids=list(range(B)))
    output = np.stack([res.results[b]["out"] for b in range(B)])
    context = np.stack([
        np.broadcast_to(res.results[b]["ctxrow"], (T, HID)).copy()
        for b in range(B)
    ])
    return output.astype(np.float32), context.astype(np.float32)
